# revision 1
# baseline (speedup 1.0000x reference)
"""Dilated (3x3, dilation=2) local-window attention for Trainium2.

Full inputs: x (32, 3136, 96) f32, W_qkv (288, 96) f32.
Sharding: data-parallel over batch, 4 images per core on 8 cores.

Per-core pipeline (per image, H=W=56, C=96, taps=9), all PE inputs bf16:
  - Fused score projection: S = K^T Q = xT^T (Wk^T Wq) xT, so only
    qk2 = (Wk^T Wq) xT is computed -- k never exists on device.
  - Row-PAIR packing: token rows (2m, 2m+1) share one matmul; scores
    S2[pos(2rows)=112, tok(2rows)=112] per (pair m, row-offset di in
    {-2,0,2}); di is even so pos rows stay row-pair aligned.
  - softmax: exp on ACT (scale folded in), multiplicative 0/1 band mask
    (also zeroing the cross-row quadrants) on DVE. exp(0)=1 rows from
    zeroed pad pairs reproduce the reference's softmax over zero-padded
    taps; w-edge taps that fall outside the unpadded 56-wide rows are
    restored by a rank-1 "zcorr" matmul into the denominator channel.
  - V computed directly token-major: lhsT = xT_ext (ones row appended)
    gives v_pair (112, 97) in PSUM with a built-in ones channel for the
    softmax denominator.
  - AV: out_pair (97, 112) accumulates 3 di matmuls + zcorr.
  - final division by the denominator channel is done on host (f32).
"""

import os
import numpy as np
import ml_dtypes

import concourse.bass as bass
import concourse.bacc as bacc
import concourse.tile as tile
from concourse import mybir
from concourse.bass_utils import run_bass_kernel_spmd

BF16 = mybir.dt.bfloat16
F32 = mybir.dt.float32

B = 32
NCORES = 8
BPC = B // NCORES  # images per core
H = 56
C = 96
N = H * H
PAD = 2
SCALE = C ** -0.5
NP = 30          # padded row pairs (rows -2..57 as 30 pairs)
M = H // 2       # 28 token row pairs per image
MG = 4           # token pairs per processing group
G = M // MG      # 7 groups
TP = 2 * H       # 112 tokens per pair
CH = MG * TP     # 448 tokens per group
DIS = (-2, 0, 2)

_NC_CACHE = {}


def build_nc():
    nc = bacc.Bacc("TRN2", target_bir_lowering=False)
    xt_d = nc.dram_tensor("xt", [BPC, C, N], BF16, kind="ExternalInput")
    wt_d = nc.dram_tensor("wt", [C, 2 * C + 1], BF16, kind="ExternalInput")
    mk_d = nc.dram_tensor("mask", [TP, 3, TP], BF16, kind="ExternalInput")
    zc_d = nc.dram_tensor("zc", [1, TP], BF16, kind="ExternalInput")
    o_d = nc.dram_tensor("o", [BPC, C + 1, N], F32, kind="ExternalOutput")

    with tile.TileContext(nc) as tc:
        _body(tc, xt_d, wt_d, mk_d, zc_d, o_d)
    nc.compile()
    return nc


def _body(tc, xt_d, wt_d, mk_d, zc_d, o_d):
    nc = tc.nc
    with (
        tc.tile_pool(name="const", bufs=1) as const,
        tc.tile_pool(name="img", bufs=2) as img,
        tc.tile_pool(name="epool", bufs=4) as epool,
        tc.tile_pool(name="osb", bufs=4) as osb,
        tc.tile_pool(name="psq", bufs=1, space="PSUM") as psq,
        tc.tile_pool(name="psv", bufs=1, space="PSUM") as psv,
        tc.tile_pool(name="pss", bufs=4, space="PSUM") as pss,
        tc.tile_pool(name="pso", bufs=2, space="PSUM") as pso,
    ):
        # wt = [wqk^T | Wv^T | unit] : cols 0:96 fused score weights,
        # 96:192 v weights, col 192 drives the denominator ones channel.
        w_sb = const.tile([C, 2 * C + 1], BF16)
        nc.sync.dma_start(w_sb[:], wt_d[:])
        # v projection rhs: (97, 97) = [[WvT, 0], [0, 1]]
        wv_ext = const.tile([C + 1, C + 1], BF16)
        nc.gpsimd.memset(wv_ext[:, :], 0.0)
        nc.vector.tensor_copy(wv_ext[0:C, 0:C], w_sb[:, C:2 * C])
        nc.vector.tensor_copy(wv_ext[C:C + 1, C:C + 1],
                              w_sb[0:1, 2 * C:2 * C + 1])
        t_sb = const.tile([TP, 3, MG, TP], BF16)
        nc.sync.dma_start(
            t_sb[:],
            bass.AP(tensor=mk_d.ap().tensor, offset=0,
                    ap=[[3 * TP, TP], [TP, 3], [0, MG], [1, TP]]))
        zc_sb = const.tile([1, TP], BF16)
        nc.sync.dma_start(zc_sb[:], zc_d[:])
        # zcorr lhsT: (1, 97) unit vector selecting the denominator row
        zl_sb = const.tile([1, C + 1], BF16)
        nc.gpsimd.memset(zl_sb[:, :], 0.0)
        nc.gpsimd.memset(zl_sb[:, C:C + 1], 1.0)
        zz_sb = const.tile([1, TP], BF16)
        nc.gpsimd.memset(zz_sb[:, :], 0.0)

        # persistent double-buffered xT / v-image tiles; invariant regions
        # (ones row, pad pairs) initialized once
        xT_bufs = [const.tile([C + 1, N], BF16, name=f"xTb{i}")
                   for i in range(2)]
        vp_bufs = [const.tile([TP, NP, C + 1], BF16, name=f"vpb{i}")
                   for i in range(2)]
        for i in range(2):
            nc.gpsimd.memset(xT_bufs[i][C:C + 1, :], 1.0)
            for rp in (0, NP - 1):
                nc.gpsimd.memset(vp_bufs[i][:, rp, 0:C], 0.0)
                nc.gpsimd.memset(vp_bufs[i][:, rp, C:C + 1], 1.0)

        for b in range(BPC):
            xT = xT_bufs[b % 2]
            nc.sync.dma_start(xT[0:C, :], xt_d[b])
            qk2 = img.tile([C, N], BF16, tag="qk2")
            vp = vp_bufs[b % 2]

            # qk2 = (Wk^T Wq) xT ; v_ext pairs = xT_ext^T Wv_ext
            for j in range(G):
                tok = slice(j * CH, (j + 1) * CH)
                pq = psq.tile([C, CH], F32, tag="pq")
                nc.tensor.matmul(pq[:], w_sb[:, 0:C], xT[0:C, tok],
                                 start=True, stop=True)
                nc.vector.tensor_copy(qk2[:, tok], pq[:])
            for m0 in range(0, M, 4):
                pv = psv.tile([TP, 4, C + 1], F32, tag="pv")
                for q in range(4):
                    m = m0 + q
                    nc.tensor.matmul(pv[:, q, :],
                                     xT[:, m * TP:(m + 1) * TP], wv_ext[:],
                                     start=True, stop=True)
                if (m0 // 4) % 2 == 0:
                    nc.vector.tensor_copy(vp[:, m0 + 1:m0 + 5, :], pv[:])
                else:
                    nc.scalar.copy(vp[:, m0 + 1:m0 + 5, :], pv[:])

            # attention, one group of 4 token pairs at a time
            for g in range(G):
                er = epool.tile([TP, 3, CH], BF16, tag="eraw")
                for di_i, di in enumerate(DIS):
                    sps = pss.tile([TP, CH], F32, tag="s")
                    for m4 in range(MG):
                        m = g * MG + m4
                        rp = m + di // 2 + 1
                        out_ap = sps[:, m4 * TP:(m4 + 1) * TP]
                        if 1 <= rp <= NP - 2:
                            nc.tensor.matmul(
                                out_ap,
                                xT[0:C, (rp - 1) * TP:rp * TP],
                                qk2[:, m * TP:(m + 1) * TP],
                                start=True, stop=True)
                        else:
                            # pad row pair: scores are 0 (zero-padded k)
                            nc.tensor.matmul(
                                out_ap, zz_sb[:, 0:TP], zz_sb[:, 0:TP],
                                start=True, stop=True)
                    nc.scalar.activation(er[:, di_i, :], sps[:],
                                         mybir.ActivationFunctionType.Exp,
                                         scale=SCALE)
                em = epool.tile([TP, 3, CH], BF16, tag="emask")
                nc.vector.tensor_mul(
                    em[:], er[:],
                    t_sb.rearrange("p d g t -> p d (g t)"))

                for k2 in range(2):
                    o_sb = osb.tile([C + 1, 2 * TP], F32, tag="osb")
                    ops = pso.tile([C + 1, 2 * TP], F32, tag="o")
                    for mm in range(2):
                        m4 = k2 * 2 + mm
                        m = g * MG + m4
                        for di_i, di in enumerate(DIS):
                            rp = m + di // 2 + 1
                            nc.tensor.matmul(
                                ops[:, mm * TP:(mm + 1) * TP],
                                vp[:, rp, :],
                                em[:, di_i, m4 * TP:(m4 + 1) * TP],
                                start=(di_i == 0), stop=False)
                        # restore denominator mass of w-out-of-range taps
                        nc.tensor.matmul(ops[:, mm * TP:(mm + 1) * TP],
                                         zl_sb[:], zc_sb[:],
                                         start=False, stop=True)
                    if k2 == 0:
                        nc.vector.tensor_copy(o_sb[:], ops[:])
                    else:
                        nc.scalar.copy(o_sb[:], ops[:])
                    m0 = (g * MG + k2 * 2) * TP
                    nc.sync.dma_start(o_d[b, :, m0:m0 + 2 * TP], o_sb[:])


def _host_consts():
    # mask (112, 3, 112): valid tap iff same row half and p-w in {-2,0,2}
    t = np.zeros((TP, 3, TP), dtype=np.float32)
    for half in range(2):
        for p in range(H):
            for w in range(H):
                if p - w in (-2, 0, 2):
                    t[half * H + p, :, half * H + w] = 1.0
    # zcorr (1, 112): 3 di-values per out-of-range dj tap
    zc = np.zeros((1, TP), dtype=np.float32)
    for half in range(2):
        for w in range(H):
            zc[0, half * H + w] = 3.0 * ((w < 2) + (w >= H - 2))
    return t.astype(ml_dtypes.bfloat16), zc.astype(ml_dtypes.bfloat16)


def kernel(x, W_qkv):
    x = np.asarray(x, dtype=np.float32)
    W_qkv = np.asarray(W_qkv, dtype=np.float32)

    if "nc" not in _NC_CACHE:
        _NC_CACHE["nc"] = build_nc()
    nc = _NC_CACHE["nc"]

    # host-side repack: shard over batch, transpose to c-major, cast bf16
    xt = np.ascontiguousarray(
        x.reshape(NCORES, BPC, N, C).transpose(0, 1, 3, 2)
    ).astype(ml_dtypes.bfloat16)
    wq = W_qkv[0:C, :]
    wk = W_qkv[C:2 * C, :]
    wv = W_qkv[2 * C:3 * C, :]
    wqk = wk.T @ wq          # (96, 96): S = xT^T wqk xT
    wt = np.zeros((C, 2 * C + 1), dtype=np.float32)
    wt[:, 0:C] = wqk.T       # lhsT for qk2 = wqk @ xT
    wt[:, C:2 * C] = wv.T
    wt[0, 2 * C] = 1.0
    wt = wt.astype(ml_dtypes.bfloat16)
    mk, zc = _host_consts()

    in_maps = [{"xt": xt[i], "wt": wt, "mask": mk, "zc": zc}
               for i in range(NCORES)]
    bkr = run_bass_kernel_spmd(nc, in_maps, list(range(NCORES)))
    _NC_CACHE["last_results"] = bkr
    res = bkr.results

    o = np.stack([np.asarray(r["o"], dtype=np.float32) for r in res])
    # o: (ncores, bpc, c+1, n) -> normalize, back to (b, n, c)
    out = o[:, :, 0:C, :] / o[:, :, C:C + 1, :]
    out = out.transpose(0, 1, 3, 2).reshape(B, N, C)
    return np.ascontiguousarray(out.astype(np.float32))

